# revision 18
# baseline (speedup 1.0000x reference)
"""Bass/Trainium2 kernel for nn_BiDirectionalCrossAttention.

Data-parallel over batch: 8 examples -> 8 NeuronCores, one example per core.

Per-core computation (L=1024, S'=2048, D=512, Kc=1280):
  tok   = conv1d(tokens) as GEMM: x(2048,1280) @ WcT(1280,512) + b   (feature-major tokT)
  R_lat = latents @ W_lat^T, R_tok = tok @ W_tok^T                    (feature-major)
  A     = R_lat @ R_tok^T / sqrt(512);  E = exp(A)
  delta_lat = rowsoftmax(A) @ V_tok ; delta_tok = colsoftmax(A)^T @ V_lat
  out   = concat(latents + delta_lat, tok + delta_tok)

All matmuls in float32r (full PE rate, ~1e-4 rel err). PE also does all
transposes (fp32 DMA transpose unsupported). Softmax denominators come free
via activation accum_out (row sums on exp, col sums on the E^T copies).
"""
import os
import sys

for _p in ("/opt/trn_rl_repo", os.path.expanduser("~/.axon_site/_ro/trn_rl_repo")):
    if os.path.isdir(_p):
        if _p not in sys.path:
            sys.path.insert(0, _p)
        break

import numpy as np  # noqa: E402

import concourse.bass as bass  # noqa: E402,F401
import concourse.tile as tile  # noqa: E402
from concourse import bacc, mybir  # noqa: E402
from concourse.bass_utils import run_bass_kernel_spmd  # noqa: E402
from concourse.masks import make_identity  # noqa: E402
from contextlib import ExitStack  # noqa: E402

P = 128
L, S2, D, KC = 1024, 2048, 512, 1280
NLT, NST, NDT, NKT = L // P, S2 // P, D // P, KC // P  # 8, 16, 4, 10
NCORES = 8
SCALE = 1.0 / np.sqrt(np.float32(D))
CH = 256              # conv s-chunk width (tokens)
NCH = S2 // CH        # 8 chunks
XPC = CH // P         # x row-tiles per chunk (2)

F32 = mybir.dt.float32
F32R = mybir.dt.float32r
Act = mybir.ActivationFunctionType

KT_GROUPS = [(0, 4), (4, 4), (8, 2)]  # kt transpose groups (start, len)


def _emit_body(nc, tc, d, top):
    """Emit one full forward pass. d = dict of dram tensors."""
    g = top.enter_context(tc.tile_pool(name="g", bufs=1))
    ident = g.tile([P, P], F32R)
    nc.sync.dma_start(ident, d["ident"][:])
    cb_sb = g.tile([P, NDT], F32)
    rs_parts = g.tile([P, NLT, 4], F32)
    recip_rs = g.tile([P, NLT], F32)
    cs_parts = g.tile([P, NST, 2], F32)
    recip_cs = g.tile([P, NST], F32)
    tokT = [g.tile([P, S2], F32R, name=f"tokT{j}") for j in range(NDT)]
    E = [g.tile([P, S2], F32R, name=f"E{lt}") for lt in range(NLT)]
    latT = g.tile([P, NDT, L], F32R)

    # wlat lives through phases 1-3 so R_lat can fill PE gaps near conv end
    with ExitStack() as mid:
        wlp = mid.enter_context(tc.tile_pool(name="wlp", bufs=1))
        wlat_sb = wlp.tile([P, NDT, D], F32R, name="wlat_sb")

        # ---------- phase 1: conv + latents transpose (interleaved) ----------
        with ExitStack() as ph1:
            wcp = ph1.enter_context(tc.tile_pool(name="wc", bufs=1))
            xrow = ph1.enter_context(tc.tile_pool(name="xrow", bufs=3))
            xtp = ph1.enter_context(tc.tile_pool(name="xt", bufs=3))
            latin = ph1.enter_context(tc.tile_pool(name="latin", bufs=2))
            ptr = ph1.enter_context(tc.tile_pool(name="ptr", bufs=3, space="PSUM"))
            pcv = ph1.enter_context(tc.tile_pool(name="pcv", bufs=3, space="PSUM"))
            ptl = ph1.enter_context(tc.tile_pool(name="ptl", bufs=2, space="PSUM"))

            wc_sb = wcp.tile([P, NKT, D], F32R)

            def emit_lat(lt):
                lin = latin.tile([P, D], F32R, tag="lin", name=f"lin{lt}")
                nc.sync.dma_start(lin, d["latr"][lt * P:(lt + 1) * P, :])
                ps = ptl.tile([P, D], F32R, tag="ptl", name=f"ptl{lt}")
                for dt in range(NDT):
                    nc.tensor.transpose(ps[:, dt * P:(dt + 1) * P],
                                        lin[:, dt * P:(dt + 1) * P], ident)
                nc.vector.tensor_copy(
                    latT[:, :, lt * P:(lt + 1) * P],
                    ps.rearrange("p (g c) -> p g c", g=NDT))

            def emit_chunk(c):
                xt = xtp.tile([P, NKT, CH], F32R, tag="xt", name=f"xt{c}")
                for i in range(XPC):
                    ti = c * XPC + i
                    xr = xrow.tile([P, KC], F32R, tag="xr", name=f"xr{ti}")
                    nc.sync.dma_start(xr, d["x"][ti * P:(ti + 1) * P, :])
                    for g0, glen in KT_GROUPS:
                        ps = ptr.tile([P, D], F32R, tag="ptr", name=f"ptr{ti}_{g0}")
                        for k in range(glen):
                            nc.tensor.transpose(ps[:, k * P:(k + 1) * P],
                                                xr[:, (g0 + k) * P:(g0 + k + 1) * P],
                                                ident)
                        nc.vector.tensor_copy(
                            xt[:, g0:g0 + glen, i * P:(i + 1) * P],
                            ps[:, :glen * P].rearrange("p (g c) -> p g c", g=glen))
                for j in range(NDT):
                    pc = pcv.tile([P, CH], F32, tag="pcv", name=f"pcv{c}_{j}")
                    for kt in range(NKT):
                        nc.tensor.matmul(pc, wc_sb[:, kt, j * P:(j + 1) * P],
                                         xt[:, kt, :],
                                         start=(kt == 0), stop=(kt == NKT - 1))
                    nc.scalar.activation(tokT[j][:, c * CH:(c + 1) * CH], pc,
                                         Act.Identity, bias=cb_sb[:, j:j + 1])

            # DMA order: lat0, x chunk0, wc, cb, lat1, chunk1, wlat, rest
            emit_lat(0)
            # chunk0 x DMAs + transposes first, conv weights next in queue
            xt0 = None
            emit_chunk_start = True
            # inline chunk0 with wc DMA issued after its x DMAs
            c = 0
            xt = xtp.tile([P, NKT, CH], F32R, tag="xt", name="xt0")
            for i in range(XPC):
                ti = i
                xr = xrow.tile([P, KC], F32R, tag="xr", name=f"xr{ti}")
                nc.sync.dma_start(xr, d["x"][ti * P:(ti + 1) * P, :])
                for g0, glen in KT_GROUPS:
                    ps = ptr.tile([P, D], F32R, tag="ptr", name=f"ptr{ti}_{g0}")
                    for k in range(glen):
                        nc.tensor.transpose(ps[:, k * P:(k + 1) * P],
                                            xr[:, (g0 + k) * P:(g0 + k + 1) * P],
                                            ident)
                    nc.vector.tensor_copy(
                        xt[:, g0:g0 + glen, i * P:(i + 1) * P],
                        ps[:, :glen * P].rearrange("p (g c) -> p g c", g=glen))
                wct_r = d["wct"].rearrange("(ko p) e -> p ko e", p=P)
                if i == 0:
                    nc.sync.dma_start(wc_sb[:, :5], wct_r[:, :5])
                else:
                    nc.sync.dma_start(wc_sb[:, 5:], wct_r[:, 5:])
                    nc.sync.dma_start(cb_sb, d["cb"][:])
            for j in range(NDT):
                pc = pcv.tile([P, CH], F32, tag="pcv", name=f"pcv0_{j}")
                for kt in range(NKT):
                    nc.tensor.matmul(pc, wc_sb[:, kt, j * P:(j + 1) * P],
                                     xt[:, kt, :],
                                     start=(kt == 0), stop=(kt == NKT - 1))
                nc.scalar.activation(tokT[j][:, 0:CH], pc,
                                     Act.Identity, bias=cb_sb[:, j:j + 1])

            emit_lat(1)
            emit_chunk(1)
            nc.sync.dma_start(wlat_sb, d["wlat"].rearrange("(ko p) e -> p ko e", p=P))
            for step in range(2, NCH):
                if step < NLT:
                    emit_lat(step)
                emit_chunk(step)

        # ---------- phases 2+3: R projections, then A+exp ----------
        with ExitStack() as ph2:
            rlp = ph2.enter_context(tc.tile_pool(name="rl", bufs=1))
            rtp = ph2.enter_context(tc.tile_pool(name="rt", bufs=1))
            RlatT = [rlp.tile([P, L], F32R, name=f"RlatT{j}") for j in range(NDT)]
            RtokT = [rtp.tile([P, S2], F32R, name=f"RtokT{j}") for j in range(NDT)]

            with ExitStack() as phw:
                wrp = phw.enter_context(tc.tile_pool(name="wr", bufs=1))
                pr = phw.enter_context(tc.tile_pool(name="pr", bufs=3, space="PSUM"))
                wtok_sb = wrp.tile([P, NDT, D], F32R, name="wtok_sb")
                nc.sync.dma_start(wtok_sb, d["wtok"].rearrange("(ko p) e -> p ko e", p=P))
                for j in range(NDT):
                    for h in range(L // D):
                        ps = pr.tile([P, D], F32, tag="pr", name=f"prl{j}_{h}")
                        for dt in range(NDT):
                            nc.tensor.matmul(ps, wlat_sb[:, dt, j * P:(j + 1) * P],
                                             latT[:, dt, h * D:(h + 1) * D],
                                             start=(dt == 0), stop=(dt == NDT - 1))
                        nc.vector.tensor_copy(RlatT[j][:, h * D:(h + 1) * D], ps)
                for j in range(NDT):
                    for c in range(4):
                        ps = pr.tile([P, D], F32, tag="pr", name=f"prt{j}_{c}")
                        for dt in range(NDT):
                            nc.tensor.matmul(ps, wtok_sb[:, dt, j * P:(j + 1) * P],
                                             tokT[dt][:, c * D:(c + 1) * D],
                                             start=(dt == 0), stop=(dt == NDT - 1))
                        nc.vector.tensor_copy(RtokT[j][:, c * D:(c + 1) * D], ps)

            with ExitStack() as pha:
                pa = pha.enter_context(tc.tile_pool(name="pa", bufs=3, space="PSUM"))
                for lt in range(NLT):
                    for c in range(4):
                        ps = pa.tile([P, D], F32, tag="pa", name=f"pa{lt}_{c}")
                        for j in range(NDT):
                            nc.tensor.matmul(ps, RlatT[j][:, lt * P:(lt + 1) * P],
                                             RtokT[j][:, c * D:(c + 1) * D],
                                             start=(j == 0), stop=(j == NDT - 1))
                        nc.scalar.activation(E[lt][:, c * D:(c + 1) * D], ps,
                                             Act.Exp, scale=float(SCALE),
                                             accum_out=rs_parts[:, lt, c:c + 1])

    for lt in range(NLT):
        nc.vector.reduce_sum(recip_rs[:, lt:lt + 1], rs_parts[:, lt, :],
                             axis=mybir.AxisListType.X)
    nc.vector.reciprocal(recip_rs, recip_rs)

    # ---------- phase 4: V projections ----------
    vpool = top.enter_context(tc.tile_pool(name="vpool", bufs=1))
    Vlat = [vpool.tile([P, D], F32R, name=f"Vlat{lt}") for lt in range(NLT)]
    Vtok = [vpool.tile([P, D], F32R, name=f"Vtok{st}") for st in range(NST)]
    e2p = top.enter_context(tc.tile_pool(name="e2", bufs=4))
    pe2 = top.enter_context(tc.tile_pool(name="pe2", bufs=2, space="PSUM"))
    with ExitStack() as ph4:
        wvp = ph4.enter_context(tc.tile_pool(name="wv", bufs=1))
        pv = ph4.enter_context(tc.tile_pool(name="pv", bufs=3, space="PSUM"))
        wvlat_sb = wvp.tile([P, NDT, D], F32R, name="wvlat_sb")
        wvtok_sb = wvp.tile([P, NDT, D], F32R, name="wvtok_sb")
        nc.sync.dma_start(wvlat_sb, d["wvlat"].rearrange("(ko p) e -> p ko e", p=P))
        nc.sync.dma_start(wvtok_sb, d["wvtok"].rearrange("(ko p) e -> p ko e", p=P))
        for lt in range(NLT):
            ps = pv.tile([P, D], F32, tag="pv", name=f"pvl{lt}")
            for dt in range(NDT):
                nc.tensor.matmul(ps, latT[:, dt, lt * P:(lt + 1) * P],
                                 wvlat_sb[:, dt, :],
                                 start=(dt == 0), stop=(dt == NDT - 1))
            nc.scalar.copy(Vlat[lt], ps)
        for st in range(NST):
            ps = pv.tile([P, D], F32, tag="pv", name=f"pvt{st}")
            for dt in range(NDT):
                nc.tensor.matmul(ps, tokT[dt][:, st * P:(st + 1) * P],
                                 wvtok_sb[:, dt, :],
                                 start=(dt == 0), stop=(dt == NDT - 1))
            nc.scalar.copy(Vtok[st], ps)

    # ---------- phase 5: delta sweeps ----------
    with ExitStack() as ph5:
        tmpp = ph5.enter_context(tc.tile_pool(name="tmp", bufs=3))
        outp = ph5.enter_context(tc.tile_pool(name="outt", bufs=4))
        latf = ph5.enter_context(tc.tile_pool(name="latf", bufs=4))
        pdl = ph5.enter_context(tc.tile_pool(name="pdl", bufs=4, space="PSUM"))
        pdt = ph5.enter_context(tc.tile_pool(name="pdt", bufs=1, space="PSUM"))
        ptk = ph5.enter_context(tc.tile_pool(name="ptk", bufs=1, space="PSUM"))

        for sweep in range(2):
            lts = list(range(4)) if sweep == 0 else list(range(4, 8))
            dl_ps = [pdl.tile([P, D], F32, tag="pdl", name=f"dl{sweep}_{k}")
                     for k in range(4)]
            lf_tiles = {}
            for lt in lts:
                lf = latf.tile([P, D], F32, tag="latf", name=f"lf{lt}")
                nc.sync.dma_start(lf, d["lat"][lt * P:(lt + 1) * P, :])
                lf_tiles[lt] = lf
            for st in range(NST):
                pst = pe2.tile([P, D], F32R, tag="pe2", name=f"pe2_{sweep}_{st}")
                for k, lt in enumerate(lts):
                    nc.tensor.transpose(pst[:, k * P:(k + 1) * P],
                                        E[lt][:, st * P:(st + 1) * P], ident)
                e2 = e2p.tile([P, D], F32R, tag="e2", name=f"e2_{sweep}_{st}")
                nc.scalar.activation(e2, pst, Act.Copy,
                                     accum_out=cs_parts[:, st, sweep:sweep + 1])
                if sweep == 0:
                    for k in range(4):
                        nc.tensor.matmul(dl_ps[k], e2[:, k * P:(k + 1) * P], Vtok[st],
                                         start=(st == 0), stop=(st == NST - 1))
                if sweep == 1 and st == NST - 1:
                    for k in range(4):
                        nc.tensor.matmul(dl_ps[k], e2[:, k * P:(k + 1) * P], Vtok[st],
                                         start=(st == 0), stop=True)
                if sweep == 1:
                    nc.vector.reduce_sum(recip_cs[:, st:st + 1],
                                         cs_parts[:, st, :],
                                         axis=mybir.AxisListType.X)
                    nc.vector.reciprocal(recip_cs[:, st:st + 1],
                                         recip_cs[:, st:st + 1])
                    dt_ps = pdt.tile([P, D], F32, tag="pdt", name=f"dt{st}")
                    for lt in range(NLT):
                        nc.tensor.matmul(dt_ps, E[lt][:, st * P:(st + 1) * P],
                                         Vlat[lt],
                                         start=(lt == 0), stop=(lt == NLT - 1))
                    tk_ps = ptk.tile([P, D], F32R, tag="ptk", name=f"tk{st}")
                    for j in range(NDT):
                        nc.tensor.transpose(tk_ps[:, j * P:(j + 1) * P],
                                            tokT[j][:, st * P:(st + 1) * P], ident)
                    tmp = tmpp.tile([P, D], F32, tag="tmp", name=f"tmpu{st}")
                    nc.scalar.activation(tmp, dt_ps, Act.Copy,
                                         scale=recip_cs[:, st:st + 1])
                    ut = outp.tile([P, D], F32, tag="outt", name=f"ut{st}")
                    nc.vector.tensor_add(ut, tmp, tk_ps)
                    nc.sync.dma_start(d["out"][L + st * P:L + (st + 1) * P, :], ut)
                    if st < NST - 1:
                        for k in range(4):
                            nc.tensor.matmul(dl_ps[k], e2[:, k * P:(k + 1) * P],
                                             Vtok[st],
                                             start=(st == 0), stop=False)
            for k, lt in enumerate(lts):
                tmp = tmpp.tile([P, D], F32, tag="tmp", name=f"tmpl{lt}")
                nc.scalar.activation(tmp, dl_ps[k], Act.Copy,
                                     scale=recip_rs[:, lt:lt + 1])
                ul = outp.tile([P, D], F32, tag="outt", name=f"ul{lt}")
                nc.vector.tensor_add(ul, tmp, lf_tiles[lt])
                nc.sync.dma_start(d["out"][lt * P:(lt + 1) * P, :], ul)


def _build_nc(reps=1):
    nc = bacc.Bacc("TRN2", target_bir_lowering=False)
    d = {
        "x": nc.dram_tensor("x", (S2, KC), F32R, kind="ExternalInput"),
        "latr": nc.dram_tensor("latents_r", (L, D), F32R, kind="ExternalInput"),
        "lat": nc.dram_tensor("latents", (L, D), F32, kind="ExternalInput"),
        "wct": nc.dram_tensor("wct", (KC, D), F32R, kind="ExternalInput"),
        "wlat": nc.dram_tensor("wlat", (D, D), F32R, kind="ExternalInput"),
        "wtok": nc.dram_tensor("wtok", (D, D), F32R, kind="ExternalInput"),
        "wvlat": nc.dram_tensor("wvlat", (D, D), F32R, kind="ExternalInput"),
        "wvtok": nc.dram_tensor("wvtok", (D, D), F32R, kind="ExternalInput"),
        "cb": nc.dram_tensor("cb", (P, NDT), F32, kind="ExternalInput"),
        "ident": nc.dram_tensor("ident", (P, P), F32R, kind="ExternalInput"),
        "out": nc.dram_tensor("out", (L + S2, D), F32, kind="ExternalOutput"),
    }
    with tile.TileContext(nc) as tc:
        if reps == 1:
            with ExitStack() as top:
                _emit_body(nc, tc, d, top)
        else:
            with tc.For_i(0, reps, 1):
                with ExitStack() as top:
                    _emit_body(nc, tc, d, top)
    nc.compile()
    return nc


_CACHE = {}


def _get_nc(reps=1):
    key = ("nc", reps)
    if key not in _CACHE:
        _CACHE[key] = _build_nc(reps)
    return _CACHE[key]


def kernel(latents, tokens, W_lat, W_tok, W_vlat, W_vtok, conv_w, conv_b):
    latents = np.ascontiguousarray(np.asarray(latents, dtype=np.float32))
    tokens = np.ascontiguousarray(np.asarray(tokens, dtype=np.float32))
    B = latents.shape[0]
    assert B == NCORES and latents.shape == (B, L, D) and tokens.shape == (B, S2 * 5, 256)

    wct = np.ascontiguousarray(np.asarray(conv_w, np.float32).transpose(2, 1, 0).reshape(KC, D))
    wlat = np.ascontiguousarray(np.asarray(W_lat, np.float32).T)
    wtok = np.ascontiguousarray(np.asarray(W_tok, np.float32).T)
    wvlat = np.ascontiguousarray(np.asarray(W_vlat, np.float32).T)
    wvtok = np.ascontiguousarray(np.asarray(W_vtok, np.float32).T)
    cb = np.ascontiguousarray(np.asarray(conv_b, np.float32).reshape(NDT, P).T)
    ident128 = np.eye(P, dtype=np.float32)

    nc = _get_nc()
    in_maps = []
    for b in range(B):
        xb = np.ascontiguousarray(tokens[b].reshape(S2, KC))
        in_maps.append({
            "x": xb,
            "latents_r": latents[b],
            "latents": latents[b],
            "wct": wct,
            "wlat": wlat,
            "wtok": wtok,
            "wvlat": wvlat,
            "wvtok": wvtok,
            "cb": cb,
            "ident": ident128,
        })
    res = run_bass_kernel_spmd(nc, in_maps, core_ids=list(range(NCORES)))
    out = np.stack([res.results[b]["out"] for b in range(B)])
    updated_latents = out[:, :L, :]
    updated_tokens = out[:, L:, :]
    return (updated_latents, updated_tokens, out)


# revision 23
# speedup vs baseline: 1.0425x; 1.0425x over previous
"""Bass/Trainium2 kernel for nn_BiDirectionalCrossAttention.

Data-parallel over batch: 8 examples -> 8 NeuronCores, one example per core.

Per-core computation (L=1024, S'=2048, D=512, Kc=1280):
  tok   = conv1d(tokens) as GEMM: x(2048,1280) @ WcT(1280,512) + b   (feature-major tokT)
  R_lat = latents @ W_lat^T, R_tok = tok @ W_tok^T                    (feature-major)
  A     = R_lat @ R_tok^T / sqrt(512);  E = exp(A)
  delta_lat = rowsoftmax(A) @ V_tok ; delta_tok = colsoftmax(A)^T @ V_lat
  out   = concat(latents + delta_lat, tok + delta_tok)

All matmuls in float32r (full PE rate, ~1e-4 rel err). PE also does all
transposes (fp32 DMA transpose unsupported). Softmax denominators come free
via activation accum_out (row sums on exp, col sums on the E^T copies).
"""
import os
import sys

for _p in ("/opt/trn_rl_repo", os.path.expanduser("~/.axon_site/_ro/trn_rl_repo")):
    if os.path.isdir(_p):
        if _p not in sys.path:
            sys.path.insert(0, _p)
        break

import numpy as np  # noqa: E402

import concourse.tile as tile  # noqa: E402
from concourse import bacc, mybir  # noqa: E402
from concourse.bass_utils import run_bass_kernel_spmd  # noqa: E402
from contextlib import ExitStack  # noqa: E402

P = 128
L, S2, D, KC = 1024, 2048, 512, 1280
NLT, NST, NDT, NKT = L // P, S2 // P, D // P, KC // P  # 8, 16, 4, 10
NCORES = 8
SCALE = 1.0 / np.sqrt(np.float32(D))
CH = 256              # conv s-chunk width (tokens)
NCH = S2 // CH        # 8 chunks
XPC = CH // P         # x row-tiles per chunk (2)

F32 = mybir.dt.float32
F32R = mybir.dt.float32r
Act = mybir.ActivationFunctionType

KT_GROUPS = [(0, 4), (4, 4), (8, 2)]  # kt transpose groups (start, len)


def _emit_body(nc, tc, d, top):
    """Emit one full forward pass. d = dict of dram tensors."""
    g = top.enter_context(tc.tile_pool(name="g", bufs=1))
    ident = g.tile([P, P], F32R)
    nc.sync.dma_start(ident, d["ident"][:])
    cb_sb = g.tile([P, NDT], F32)
    rs_parts = g.tile([P, NLT, 4], F32)
    recip_rs = g.tile([P, NLT], F32)
    cs_parts = g.tile([P, NST, 2], F32)
    recip_cs = g.tile([P, NST], F32)
    tokT = [g.tile([P, S2], F32R, name=f"tokT{j}") for j in range(NDT)]
    E = [g.tile([P, S2], F32R, name=f"E{lt}") for lt in range(NLT)]
    latT = g.tile([P, NDT, L], F32R)

    # wlat lives through phases 1-3 so R_lat can fill PE gaps near conv end
    with ExitStack() as mid:
        wlp = mid.enter_context(tc.tile_pool(name="wlp", bufs=1))
        wlat_sb = wlp.tile([P, NDT, D], F32R, name="wlat_sb")

        # ---------- phase 1: conv + latents transpose (interleaved) ----------
        with ExitStack() as ph1:
            wcp = ph1.enter_context(tc.tile_pool(name="wc", bufs=1))
            xrow = ph1.enter_context(tc.tile_pool(name="xrow", bufs=3))
            xtp = ph1.enter_context(tc.tile_pool(name="xt", bufs=3))
            latin = ph1.enter_context(tc.tile_pool(name="latin", bufs=2))
            ptr = ph1.enter_context(tc.tile_pool(name="ptr", bufs=3, space="PSUM"))
            pcv = ph1.enter_context(tc.tile_pool(name="pcv", bufs=3, space="PSUM"))
            ptl = ph1.enter_context(tc.tile_pool(name="ptl", bufs=2, space="PSUM"))

            wc_sb = wcp.tile([P, NKT, D], F32R)

            def emit_lat(lt):
                lin = latin.tile([P, D], F32R, tag="lin", name=f"lin{lt}")
                nc.sync.dma_start(lin, d["latr"][lt * P:(lt + 1) * P, :])
                ps = ptl.tile([P, D], F32R, tag="ptl", name=f"ptl{lt}")
                for dt in range(NDT):
                    nc.tensor.transpose(ps[:, dt * P:(dt + 1) * P],
                                        lin[:, dt * P:(dt + 1) * P], ident)
                nc.vector.tensor_copy(
                    latT[:, :, lt * P:(lt + 1) * P],
                    ps.rearrange("p (g c) -> p g c", g=NDT))

            def emit_chunk(c):
                xt = xtp.tile([P, NKT, CH], F32R, tag="xt", name=f"xt{c}")
                for i in range(XPC):
                    ti = c * XPC + i
                    xr = xrow.tile([P, KC], F32R, tag="xr", name=f"xr{ti}")
                    nc.sync.dma_start(xr, d["x"][ti * P:(ti + 1) * P, :])
                    for g0, glen in KT_GROUPS:
                        ps = ptr.tile([P, D], F32R, tag="ptr", name=f"ptr{ti}_{g0}")
                        for k in range(glen):
                            nc.tensor.transpose(ps[:, k * P:(k + 1) * P],
                                                xr[:, (g0 + k) * P:(g0 + k + 1) * P],
                                                ident)
                        nc.vector.tensor_copy(
                            xt[:, g0:g0 + glen, i * P:(i + 1) * P],
                            ps[:, :glen * P].rearrange("p (g c) -> p g c", g=glen))
                for j in range(NDT):
                    pc = pcv.tile([P, CH], F32, tag="pcv", name=f"pcv{c}_{j}")
                    for kt in range(NKT):
                        nc.tensor.matmul(pc, wc_sb[:, kt, j * P:(j + 1) * P],
                                         xt[:, kt, :],
                                         start=(kt == 0), stop=(kt == NKT - 1))
                    nc.scalar.activation(tokT[j][:, c * CH:(c + 1) * CH], pc,
                                         Act.Identity, bias=cb_sb[:, j:j + 1])

            # DMA queue order: ident, lat0, xr0, wc half1, xr1, wc half2+cb,
            # lat1, chunk1, wlat, then the steady stream. Keeps PE fed from
            # ~2us while the conv weights stream in behind the first x tiles.
            emit_lat(0)
            xt = xtp.tile([P, NKT, CH], F32R, tag="xt", name="xt0")
            for i in range(XPC):
                ti = i
                xr = xrow.tile([P, KC], F32R, tag="xr", name=f"xr{ti}")
                nc.sync.dma_start(xr, d["x"][ti * P:(ti + 1) * P, :])
                for g0, glen in KT_GROUPS:
                    ps = ptr.tile([P, D], F32R, tag="ptr", name=f"ptr{ti}_{g0}")
                    for k in range(glen):
                        nc.tensor.transpose(ps[:, k * P:(k + 1) * P],
                                            xr[:, (g0 + k) * P:(g0 + k + 1) * P],
                                            ident)
                    nc.vector.tensor_copy(
                        xt[:, g0:g0 + glen, i * P:(i + 1) * P],
                        ps[:, :glen * P].rearrange("p (g c) -> p g c", g=glen))
                wct_r = d["wct"].rearrange("(ko p) e -> p ko e", p=P)
                if i == 0:
                    nc.sync.dma_start(wc_sb[:, :5], wct_r[:, :5])
                else:
                    nc.sync.dma_start(wc_sb[:, 5:], wct_r[:, 5:])
                    nc.sync.dma_start(cb_sb, d["cb"][:])
            for j in range(NDT):
                pc = pcv.tile([P, CH], F32, tag="pcv", name=f"pcv0_{j}")
                for kt in range(NKT):
                    nc.tensor.matmul(pc, wc_sb[:, kt, j * P:(j + 1) * P],
                                     xt[:, kt, :],
                                     start=(kt == 0), stop=(kt == NKT - 1))
                nc.scalar.activation(tokT[j][:, 0:CH], pc,
                                     Act.Identity, bias=cb_sb[:, j:j + 1])

            emit_lat(1)
            emit_chunk(1)
            nc.sync.dma_start(wlat_sb, d["wlat"].rearrange("(ko p) e -> p ko e", p=P))
            for step in range(2, NCH):
                if step < NLT:
                    emit_lat(step)
                emit_chunk(step)

        # ---------- phases 2+3: R projections, then A+exp ----------
        with ExitStack() as ph2:
            rlp = ph2.enter_context(tc.tile_pool(name="rl", bufs=1))
            rtp = ph2.enter_context(tc.tile_pool(name="rt", bufs=1))
            RlatT = [rlp.tile([P, L], F32R, name=f"RlatT{j}") for j in range(NDT)]
            RtokT = [rtp.tile([P, S2], F32R, name=f"RtokT{j}") for j in range(NDT)]

            with ExitStack() as phw:
                wrp = phw.enter_context(tc.tile_pool(name="wr", bufs=1))
                pr = phw.enter_context(tc.tile_pool(name="pr", bufs=3, space="PSUM"))
                wtok_sb = wrp.tile([P, NDT, D], F32R, name="wtok_sb")
                nc.sync.dma_start(wtok_sb, d["wtok"].rearrange("(ko p) e -> p ko e", p=P))
                for j in range(NDT):
                    for h in range(L // D):
                        ps = pr.tile([P, D], F32, tag="pr", name=f"prl{j}_{h}")
                        for dt in range(NDT):
                            nc.tensor.matmul(ps, wlat_sb[:, dt, j * P:(j + 1) * P],
                                             latT[:, dt, h * D:(h + 1) * D],
                                             start=(dt == 0), stop=(dt == NDT - 1))
                        nc.vector.tensor_copy(RlatT[j][:, h * D:(h + 1) * D], ps)
                for j in range(NDT):
                    for c in range(4):
                        ps = pr.tile([P, D], F32, tag="pr", name=f"prt{j}_{c}")
                        for dt in range(NDT):
                            nc.tensor.matmul(ps, wtok_sb[:, dt, j * P:(j + 1) * P],
                                             tokT[dt][:, c * D:(c + 1) * D],
                                             start=(dt == 0), stop=(dt == NDT - 1))
                        nc.vector.tensor_copy(RtokT[j][:, c * D:(c + 1) * D], ps)

            with ExitStack() as pha:
                pa = pha.enter_context(tc.tile_pool(name="pa", bufs=3, space="PSUM"))
                for lt in range(NLT):
                    for c in range(4):
                        ps = pa.tile([P, D], F32, tag="pa", name=f"pa{lt}_{c}")
                        for j in range(NDT):
                            nc.tensor.matmul(ps, RlatT[j][:, lt * P:(lt + 1) * P],
                                             RtokT[j][:, c * D:(c + 1) * D],
                                             start=(j == 0), stop=(j == NDT - 1))
                        nc.scalar.activation(E[lt][:, c * D:(c + 1) * D], ps,
                                             Act.Exp, scale=float(SCALE),
                                             accum_out=rs_parts[:, lt, c:c + 1])

    for lt in range(NLT):
        nc.vector.reduce_sum(recip_rs[:, lt:lt + 1], rs_parts[:, lt, :],
                             axis=mybir.AxisListType.X)
    nc.vector.reciprocal(recip_rs, recip_rs)

    # ---------- phase 4: V projections ----------
    vpool = top.enter_context(tc.tile_pool(name="vpool", bufs=1))
    Vlat = [vpool.tile([P, D], F32R, name=f"Vlat{lt}") for lt in range(NLT)]
    Vtok = [vpool.tile([P, D], F32R, name=f"Vtok{st}") for st in range(NST)]
    e2p = top.enter_context(tc.tile_pool(name="e2", bufs=4))
    pe2 = top.enter_context(tc.tile_pool(name="pe2", bufs=2, space="PSUM"))
    with ExitStack() as ph4:
        wvp = ph4.enter_context(tc.tile_pool(name="wv", bufs=1))
        pv = ph4.enter_context(tc.tile_pool(name="pv", bufs=3, space="PSUM"))
        wvlat_sb = wvp.tile([P, NDT, D], F32R, name="wvlat_sb")
        wvtok_sb = wvp.tile([P, NDT, D], F32R, name="wvtok_sb")
        nc.sync.dma_start(wvlat_sb, d["wvlat"].rearrange("(ko p) e -> p ko e", p=P))
        nc.sync.dma_start(wvtok_sb, d["wvtok"].rearrange("(ko p) e -> p ko e", p=P))
        for lt in range(NLT):
            ps = pv.tile([P, D], F32, tag="pv", name=f"pvl{lt}")
            for dt in range(NDT):
                nc.tensor.matmul(ps, latT[:, dt, lt * P:(lt + 1) * P],
                                 wvlat_sb[:, dt, :],
                                 start=(dt == 0), stop=(dt == NDT - 1))
            nc.vector.tensor_copy(Vlat[lt], ps)
        for st in range(NST):
            ps = pv.tile([P, D], F32, tag="pv", name=f"pvt{st}")
            for dt in range(NDT):
                nc.tensor.matmul(ps, tokT[dt][:, st * P:(st + 1) * P],
                                 wvtok_sb[:, dt, :],
                                 start=(dt == 0), stop=(dt == NDT - 1))
            nc.vector.tensor_copy(Vtok[st], ps)

    # ---------- phase 5: delta sweeps ----------
    with ExitStack() as ph5:
        outp = ph5.enter_context(tc.tile_pool(name="outt", bufs=4))
        toksb = ph5.enter_context(tc.tile_pool(name="toksb", bufs=2))
        latf = ph5.enter_context(tc.tile_pool(name="latf", bufs=4))
        pdl = ph5.enter_context(tc.tile_pool(name="pdl", bufs=4, space="PSUM"))
        pdt = ptk = None

        for sweep in range(2):
            if sweep == 0:
                sw_cm = tc.tile_pool(name="pe2x", bufs=1, space="PSUM")
                pe2x = sw_cm.__enter__()
                pst_pools = [pe2, pe2x]
            else:
                sw_cm.__exit__(None, None, None)
                pdt = ph5.enter_context(tc.tile_pool(name="pdt", bufs=1, space="PSUM"))
                ptk = ph5.enter_context(tc.tile_pool(name="ptk", bufs=1, space="PSUM"))
                pst_pools = [pe2]
            lts = list(range(4)) if sweep == 0 else list(range(4, 8))
            dl_ps = [pdl.tile([P, D], F32, tag="pdl", name=f"dl{sweep}_{k}")
                     for k in range(4)]
            lf_tiles = {}
            for lt in lts:
                lf = latf.tile([P, D], F32, tag="latf", name=f"lf{lt}")
                nc.sync.dma_start(lf, d["lat"][lt * P:(lt + 1) * P, :])
                lf_tiles[lt] = lf
            for st in range(NST):
                pool_ = pst_pools[st % len(pst_pools)]
                pst = pool_.tile([P, D], F32R, tag=f"pe2{st % len(pst_pools)}",
                                 name=f"pe2_{sweep}_{st}")
                for k, lt in enumerate(lts):
                    nc.tensor.transpose(pst[:, k * P:(k + 1) * P],
                                        E[lt][:, st * P:(st + 1) * P], ident)
                e2 = e2p.tile([P, D], F32R, tag="e2", name=f"e2_{sweep}_{st}")
                nc.scalar.activation(e2, pst, Act.Copy,
                                     accum_out=cs_parts[:, st, sweep:sweep + 1])
                if sweep == 0:
                    for k in range(4):
                        nc.tensor.matmul(dl_ps[k], e2[:, k * P:(k + 1) * P], Vtok[st],
                                         start=(st == 0), stop=(st == NST - 1))
                if sweep == 1 and st == NST - 1:
                    for k in range(4):
                        nc.tensor.matmul(dl_ps[k], e2[:, k * P:(k + 1) * P], Vtok[st],
                                         start=(st == 0), stop=True)
                if sweep == 1:
                    nc.vector.reduce_sum(recip_cs[:, st:st + 1],
                                         cs_parts[:, st, :],
                                         axis=mybir.AxisListType.X)
                    nc.vector.reciprocal(recip_cs[:, st:st + 1],
                                         recip_cs[:, st:st + 1])
                    dt_ps = pdt.tile([P, D], F32, tag="pdt", name=f"dt{st}")
                    for lt in range(NLT):
                        nc.tensor.matmul(dt_ps, E[lt][:, st * P:(st + 1) * P],
                                         Vlat[lt],
                                         start=(lt == 0), stop=(lt == NLT - 1))
                    tk_ps = ptk.tile([P, D], F32R, tag="ptk", name=f"tk{st}")
                    for j in range(NDT):
                        nc.tensor.transpose(tk_ps[:, j * P:(j + 1) * P],
                                            tokT[j][:, st * P:(st + 1) * P], ident)
                    tok_sb = toksb.tile([P, D], F32, tag="toksb", name=f"toksb{st}")
                    nc.scalar.copy(tok_sb, tk_ps)
                    ut = outp.tile([P, D], F32, tag="outt", name=f"ut{st}")
                    nc.vector.scalar_tensor_tensor(
                        ut, dt_ps, recip_cs[:, st:st + 1], tok_sb,
                        op0=mybir.AluOpType.mult, op1=mybir.AluOpType.add)
                    nc.sync.dma_start(d["out"][L + st * P:L + (st + 1) * P, :], ut)
                    if st < NST - 1:
                        for k in range(4):
                            nc.tensor.matmul(dl_ps[k], e2[:, k * P:(k + 1) * P],
                                             Vtok[st],
                                             start=(st == 0), stop=False)
            for k, lt in enumerate(lts):
                ul = outp.tile([P, D], F32, tag="outt", name=f"ul{lt}")
                nc.vector.scalar_tensor_tensor(
                    ul, dl_ps[k], recip_rs[:, lt:lt + 1], lf_tiles[lt],
                    op0=mybir.AluOpType.mult, op1=mybir.AluOpType.add)
                nc.sync.dma_start(d["out"][lt * P:(lt + 1) * P, :], ul)


def _build_nc(reps=1):
    nc = bacc.Bacc("TRN2", target_bir_lowering=False)
    d = {
        "x": nc.dram_tensor("x", (S2, KC), F32R, kind="ExternalInput"),
        "latr": nc.dram_tensor("latents_r", (L, D), F32R, kind="ExternalInput"),
        "lat": nc.dram_tensor("latents", (L, D), F32, kind="ExternalInput"),
        "wct": nc.dram_tensor("wct", (KC, D), F32R, kind="ExternalInput"),
        "wlat": nc.dram_tensor("wlat", (D, D), F32R, kind="ExternalInput"),
        "wtok": nc.dram_tensor("wtok", (D, D), F32R, kind="ExternalInput"),
        "wvlat": nc.dram_tensor("wvlat", (D, D), F32R, kind="ExternalInput"),
        "wvtok": nc.dram_tensor("wvtok", (D, D), F32R, kind="ExternalInput"),
        "cb": nc.dram_tensor("cb", (P, NDT), F32, kind="ExternalInput"),
        "ident": nc.dram_tensor("ident", (P, P), F32R, kind="ExternalInput"),
        "out": nc.dram_tensor("out", (L + S2, D), F32, kind="ExternalOutput"),
    }
    with tile.TileContext(nc) as tc:
        if reps == 1:
            with ExitStack() as top:
                _emit_body(nc, tc, d, top)
        else:
            with tc.For_i(0, reps, 1):
                with ExitStack() as top:
                    _emit_body(nc, tc, d, top)
    nc.compile()
    return nc


_CACHE = {}


def _get_nc(reps=1):
    key = ("nc", reps)
    if key not in _CACHE:
        _CACHE[key] = _build_nc(reps)
    return _CACHE[key]


def kernel(latents, tokens, W_lat, W_tok, W_vlat, W_vtok, conv_w, conv_b):
    latents = np.ascontiguousarray(np.asarray(latents, dtype=np.float32))
    tokens = np.ascontiguousarray(np.asarray(tokens, dtype=np.float32))
    B = latents.shape[0]
    assert B == NCORES and latents.shape == (B, L, D) and tokens.shape == (B, S2 * 5, 256)

    wct = np.ascontiguousarray(np.asarray(conv_w, np.float32).transpose(2, 1, 0).reshape(KC, D))
    wlat = np.ascontiguousarray(np.asarray(W_lat, np.float32).T)
    wtok = np.ascontiguousarray(np.asarray(W_tok, np.float32).T)
    wvlat = np.ascontiguousarray(np.asarray(W_vlat, np.float32).T)
    wvtok = np.ascontiguousarray(np.asarray(W_vtok, np.float32).T)
    cb = np.ascontiguousarray(np.asarray(conv_b, np.float32).reshape(NDT, P).T)
    ident128 = np.eye(P, dtype=np.float32)

    nc = _get_nc()
    in_maps = []
    for b in range(B):
        xb = np.ascontiguousarray(tokens[b].reshape(S2, KC))
        in_maps.append({
            "x": xb,
            "latents_r": latents[b],
            "latents": latents[b],
            "wct": wct,
            "wlat": wlat,
            "wtok": wtok,
            "wvlat": wvlat,
            "wvtok": wvtok,
            "cb": cb,
            "ident": ident128,
        })
    res = run_bass_kernel_spmd(nc, in_maps, core_ids=list(range(NCORES)))
    out = np.stack([res.results[b]["out"] for b in range(B)])
    updated_latents = out[:, :L, :]
    updated_tokens = out[:, L:, :]
    return (updated_latents, updated_tokens, out)


# revision 24
# speedup vs baseline: 1.0870x; 1.0427x over previous
"""Bass/Trainium2 kernel for nn_BiDirectionalCrossAttention.

Data-parallel over batch: 8 examples -> 8 NeuronCores, one example per core.

Per-core computation (L=1024, S'=2048, D=512, Kc=1280):
  tok   = conv1d(tokens) as GEMM: x(2048,1280) @ WcT(1280,512) + b   (feature-major tokT)
  R_lat = latents @ W_lat^T, R_tok = tok @ W_tok^T                    (feature-major)
  A     = R_lat @ R_tok^T / sqrt(512);  E = exp(A)
  delta_lat = rowsoftmax(A) @ V_tok ; delta_tok = colsoftmax(A)^T @ V_lat
  out   = concat(latents + delta_lat, tok + delta_tok)

All matmuls in float32r (full PE rate, ~1e-4 rel err). PE also does all
transposes (fp32 DMA transpose unsupported). Softmax denominators come free
via activation accum_out (row sums on exp, col sums on the E^T copies).
"""
import os
import sys

for _p in ("/opt/trn_rl_repo", os.path.expanduser("~/.axon_site/_ro/trn_rl_repo")):
    if os.path.isdir(_p):
        if _p not in sys.path:
            sys.path.insert(0, _p)
        break

import numpy as np  # noqa: E402

import concourse.tile as tile  # noqa: E402
from concourse import bacc, mybir  # noqa: E402
from concourse.bass_utils import run_bass_kernel_spmd  # noqa: E402
from contextlib import ExitStack  # noqa: E402

P = 128
L, S2, D, KC = 1024, 2048, 512, 1280
NLT, NST, NDT, NKT = L // P, S2 // P, D // P, KC // P  # 8, 16, 4, 10
NCORES = 8
SCALE = 1.0 / np.sqrt(np.float32(D))
CH = 256              # conv s-chunk width (tokens)
NCH = S2 // CH        # 8 chunks
XPC = CH // P         # x row-tiles per chunk (2)

F32 = mybir.dt.float32
F32R = mybir.dt.float32r
Act = mybir.ActivationFunctionType

KT_GROUPS = [(0, 4), (4, 4), (8, 2)]  # kt transpose groups (start, len)


def _emit_body(nc, tc, d, top):
    """Emit one full forward pass. d = dict of dram tensors."""
    g = top.enter_context(tc.tile_pool(name="g", bufs=1))
    ident = g.tile([P, P], F32R)
    nc.sync.dma_start(ident, d["ident"][:])
    cb_sb = g.tile([P, NDT], F32)
    rs_parts = g.tile([P, NLT, 4], F32)
    recip_rs = g.tile([P, NLT], F32)
    cs_parts = g.tile([P, NST, 2], F32)
    recip_cs = g.tile([P, NST], F32)
    tokT = [g.tile([P, S2], F32R, name=f"tokT{j}") for j in range(NDT)]
    E = [g.tile([P, S2], F32R, name=f"E{lt}") for lt in range(NLT)]
    latT = g.tile([P, NDT, L], F32R)

    # wlat lives through phases 1-3 so R_lat can fill PE gaps near conv end
    with ExitStack() as mid:
        wlp = mid.enter_context(tc.tile_pool(name="wlp", bufs=1))
        wlat_sb = wlp.tile([P, NDT, D], F32R, name="wlat_sb")

        # ---------- phase 1: conv + latents transpose (interleaved) ----------
        with ExitStack() as ph1:
            wcp = ph1.enter_context(tc.tile_pool(name="wc", bufs=1))
            xrow = ph1.enter_context(tc.tile_pool(name="xrow", bufs=3))
            xtp = ph1.enter_context(tc.tile_pool(name="xt", bufs=3))
            latin = ph1.enter_context(tc.tile_pool(name="latin", bufs=2))
            ptr = ph1.enter_context(tc.tile_pool(name="ptr", bufs=3, space="PSUM"))
            pcv = ph1.enter_context(tc.tile_pool(name="pcv", bufs=3, space="PSUM"))
            ptl = ph1.enter_context(tc.tile_pool(name="ptl", bufs=2, space="PSUM"))

            wc_sb = wcp.tile([P, NKT, D], F32R)

            def emit_lat(lt):
                lin = latin.tile([P, D], F32R, tag="lin", name=f"lin{lt}")
                nc.sync.dma_start(lin, d["latr"][lt * P:(lt + 1) * P, :])
                ps = ptl.tile([P, D], F32R, tag="ptl", name=f"ptl{lt}")
                for dt in range(NDT):
                    nc.tensor.transpose(ps[:, dt * P:(dt + 1) * P],
                                        lin[:, dt * P:(dt + 1) * P], ident)
                nc.vector.tensor_copy(
                    latT[:, :, lt * P:(lt + 1) * P],
                    ps.rearrange("p (g c) -> p g c", g=NDT))

            def emit_chunk(c):
                xt = xtp.tile([P, NKT, CH], F32R, tag="xt", name=f"xt{c}")
                for i in range(XPC):
                    ti = c * XPC + i
                    xr = xrow.tile([P, KC], F32R, tag="xr", name=f"xr{ti}")
                    nc.sync.dma_start(xr, d["x"][ti * P:(ti + 1) * P, :])
                    for g0, glen in KT_GROUPS:
                        ps = ptr.tile([P, D], F32R, tag="ptr", name=f"ptr{ti}_{g0}")
                        for k in range(glen):
                            nc.tensor.transpose(ps[:, k * P:(k + 1) * P],
                                                xr[:, (g0 + k) * P:(g0 + k + 1) * P],
                                                ident)
                        nc.vector.tensor_copy(
                            xt[:, g0:g0 + glen, i * P:(i + 1) * P],
                            ps[:, :glen * P].rearrange("p (g c) -> p g c", g=glen))
                for j in range(NDT):
                    pc = pcv.tile([P, CH], F32, tag="pcv", name=f"pcv{c}_{j}")
                    for kt in range(NKT):
                        nc.tensor.matmul(pc, wc_sb[:, kt, j * P:(j + 1) * P],
                                         xt[:, kt, :],
                                         start=(kt == 0), stop=(kt == NKT - 1))
                    nc.scalar.activation(tokT[j][:, c * CH:(c + 1) * CH], pc,
                                         Act.Identity, bias=cb_sb[:, j:j + 1])

            # DMA queue order: ident, lat0, xr0, wc half1, xr1, wc half2+cb,
            # lat1, chunk1, wlat, then the steady stream. Keeps PE fed from
            # ~2us while the conv weights stream in behind the first x tiles.
            emit_lat(0)
            xt = xtp.tile([P, NKT, CH], F32R, tag="xt", name="xt0")
            for i in range(XPC):
                ti = i
                xr = xrow.tile([P, KC], F32R, tag="xr", name=f"xr{ti}")
                nc.sync.dma_start(xr, d["x"][ti * P:(ti + 1) * P, :])
                for g0, glen in KT_GROUPS:
                    ps = ptr.tile([P, D], F32R, tag="ptr", name=f"ptr{ti}_{g0}")
                    for k in range(glen):
                        nc.tensor.transpose(ps[:, k * P:(k + 1) * P],
                                            xr[:, (g0 + k) * P:(g0 + k + 1) * P],
                                            ident)
                    nc.vector.tensor_copy(
                        xt[:, g0:g0 + glen, i * P:(i + 1) * P],
                        ps[:, :glen * P].rearrange("p (g c) -> p g c", g=glen))
                wct_r = d["wct"].rearrange("(ko p) e -> p ko e", p=P)
                if i == 0:
                    nc.sync.dma_start(wc_sb[:, :5], wct_r[:, :5])
                else:
                    nc.sync.dma_start(wc_sb[:, 5:], wct_r[:, 5:])
                    nc.sync.dma_start(cb_sb, d["cb"][:])
            for j in range(NDT):
                pc = pcv.tile([P, CH], F32, tag="pcv", name=f"pcv0_{j}")
                for kt in range(NKT):
                    nc.tensor.matmul(pc, wc_sb[:, kt, j * P:(j + 1) * P],
                                     xt[:, kt, :],
                                     start=(kt == 0), stop=(kt == NKT - 1))
                nc.scalar.activation(tokT[j][:, 0:CH], pc,
                                     Act.Identity, bias=cb_sb[:, j:j + 1])

            emit_lat(1)
            emit_chunk(1)
            nc.sync.dma_start(wlat_sb, d["wlat"].rearrange("(ko p) e -> p ko e", p=P))
            lat_next = 2
            for step in range(2, NCH):
                emit_chunk(step)
                while lat_next < NLT and lat_next <= step * 2 - 2:
                    emit_lat(lat_next)
                    lat_next += 1
            while lat_next < NLT:
                emit_lat(lat_next)
                lat_next += 1

        # ---------- phases 2+3: R projections, then A+exp ----------
        with ExitStack() as ph2:
            rlp = ph2.enter_context(tc.tile_pool(name="rl", bufs=1))
            rtp = ph2.enter_context(tc.tile_pool(name="rt", bufs=1))
            RlatT = [rlp.tile([P, L], F32R, name=f"RlatT{j}") for j in range(NDT)]
            RtokT = [rtp.tile([P, S2], F32R, name=f"RtokT{j}") for j in range(NDT)]

            with ExitStack() as phw:
                wrp = phw.enter_context(tc.tile_pool(name="wr", bufs=1))
                pr = phw.enter_context(tc.tile_pool(name="pr", bufs=3, space="PSUM"))
                wtok_sb = wrp.tile([P, NDT, D], F32R, name="wtok_sb")
                nc.sync.dma_start(wtok_sb, d["wtok"].rearrange("(ko p) e -> p ko e", p=P))
                for j in range(NDT):
                    for h in range(L // D):
                        ps = pr.tile([P, D], F32, tag="pr", name=f"prl{j}_{h}")
                        for dt in range(NDT):
                            nc.tensor.matmul(ps, wlat_sb[:, dt, j * P:(j + 1) * P],
                                             latT[:, dt, h * D:(h + 1) * D],
                                             start=(dt == 0), stop=(dt == NDT - 1))
                        nc.vector.tensor_copy(RlatT[j][:, h * D:(h + 1) * D], ps)
                for j in range(NDT):
                    for c in range(4):
                        ps = pr.tile([P, D], F32, tag="pr", name=f"prt{j}_{c}")
                        for dt in range(NDT):
                            nc.tensor.matmul(ps, wtok_sb[:, dt, j * P:(j + 1) * P],
                                             tokT[dt][:, c * D:(c + 1) * D],
                                             start=(dt == 0), stop=(dt == NDT - 1))
                        nc.vector.tensor_copy(RtokT[j][:, c * D:(c + 1) * D], ps)

            with ExitStack() as pha:
                pa = pha.enter_context(tc.tile_pool(name="pa", bufs=3, space="PSUM"))
                for lt in range(NLT):
                    for c in range(4):
                        ps = pa.tile([P, D], F32, tag="pa", name=f"pa{lt}_{c}")
                        for j in range(NDT):
                            nc.tensor.matmul(ps, RlatT[j][:, lt * P:(lt + 1) * P],
                                             RtokT[j][:, c * D:(c + 1) * D],
                                             start=(j == 0), stop=(j == NDT - 1))
                        nc.scalar.activation(E[lt][:, c * D:(c + 1) * D], ps,
                                             Act.Exp, scale=float(SCALE),
                                             accum_out=rs_parts[:, lt, c:c + 1])

    for lt in range(NLT):
        nc.vector.reduce_sum(recip_rs[:, lt:lt + 1], rs_parts[:, lt, :],
                             axis=mybir.AxisListType.X)
    nc.vector.reciprocal(recip_rs, recip_rs)

    # ---------- phase 4: V projections ----------
    vpool = top.enter_context(tc.tile_pool(name="vpool", bufs=1))
    Vlat = [vpool.tile([P, D], F32R, name=f"Vlat{lt}") for lt in range(NLT)]
    Vtok = [vpool.tile([P, D], F32R, name=f"Vtok{st}") for st in range(NST)]
    e2p = top.enter_context(tc.tile_pool(name="e2", bufs=6))
    pe2 = top.enter_context(tc.tile_pool(name="pe2", bufs=2, space="PSUM"))
    with ExitStack() as ph4:
        wvp = ph4.enter_context(tc.tile_pool(name="wv", bufs=1))
        pv = ph4.enter_context(tc.tile_pool(name="pv", bufs=4, space="PSUM"))
        wvlat_sb = wvp.tile([P, NDT, D], F32R, name="wvlat_sb")
        wvtok_sb = wvp.tile([P, NDT, D], F32R, name="wvtok_sb")
        nc.sync.dma_start(wvlat_sb, d["wvlat"].rearrange("(ko p) e -> p ko e", p=P))
        nc.sync.dma_start(wvtok_sb, d["wvtok"].rearrange("(ko p) e -> p ko e", p=P))
        for lt in range(NLT):
            ps = pv.tile([P, D], F32, tag="pv", name=f"pvl{lt}")
            for dt in range(NDT):
                nc.tensor.matmul(ps, latT[:, dt, lt * P:(lt + 1) * P],
                                 wvlat_sb[:, dt, :],
                                 start=(dt == 0), stop=(dt == NDT - 1))
            nc.vector.tensor_copy(Vlat[lt], ps)
        for st in range(NST):
            ps = pv.tile([P, D], F32, tag="pv", name=f"pvt{st}")
            for dt in range(NDT):
                nc.tensor.matmul(ps, tokT[dt][:, st * P:(st + 1) * P],
                                 wvtok_sb[:, dt, :],
                                 start=(dt == 0), stop=(dt == NDT - 1))
            nc.vector.tensor_copy(Vtok[st], ps)

    # ---------- phase 5: delta sweeps ----------
    with ExitStack() as ph5:
        outp = ph5.enter_context(tc.tile_pool(name="outt", bufs=4))
        toksb = ph5.enter_context(tc.tile_pool(name="toksb", bufs=2))
        latf = ph5.enter_context(tc.tile_pool(name="latf", bufs=4))
        pdl = ph5.enter_context(tc.tile_pool(name="pdl", bufs=4, space="PSUM"))
        pdt = ptk = None

        for sweep in range(2):
            if sweep == 0:
                sw_cm = tc.tile_pool(name="pe2x", bufs=1, space="PSUM")
                pe2x = sw_cm.__enter__()
                pst_pools = [pe2, pe2x]
            else:
                sw_cm.__exit__(None, None, None)
                pdt = ph5.enter_context(tc.tile_pool(name="pdt", bufs=1, space="PSUM"))
                ptk = ph5.enter_context(tc.tile_pool(name="ptk", bufs=1, space="PSUM"))
                pst_pools = [pe2]
            lts = list(range(4)) if sweep == 0 else list(range(4, 8))
            dl_ps = [pdl.tile([P, D], F32, tag="pdl", name=f"dl{sweep}_{k}")
                     for k in range(4)]
            lf_tiles = {}
            for lt in lts:
                lf = latf.tile([P, D], F32, tag="latf", name=f"lf{lt}")
                nc.sync.dma_start(lf, d["lat"][lt * P:(lt + 1) * P, :])
                lf_tiles[lt] = lf
            for st in range(NST):
                pool_ = pst_pools[st % len(pst_pools)]
                pst = pool_.tile([P, D], F32R, tag=f"pe2{st % len(pst_pools)}",
                                 name=f"pe2_{sweep}_{st}")
                for k, lt in enumerate(lts):
                    nc.tensor.transpose(pst[:, k * P:(k + 1) * P],
                                        E[lt][:, st * P:(st + 1) * P], ident)
                e2 = e2p.tile([P, D], F32R, tag="e2", name=f"e2_{sweep}_{st}")
                nc.scalar.activation(e2, pst, Act.Copy,
                                     accum_out=cs_parts[:, st, sweep:sweep + 1])
                if sweep == 0:
                    for k in range(4):
                        nc.tensor.matmul(dl_ps[k], e2[:, k * P:(k + 1) * P], Vtok[st],
                                         start=(st == 0), stop=(st == NST - 1))
                if sweep == 1 and st == NST - 1:
                    for k in range(4):
                        nc.tensor.matmul(dl_ps[k], e2[:, k * P:(k + 1) * P], Vtok[st],
                                         start=(st == 0), stop=True)
                if sweep == 1:
                    nc.vector.reduce_sum(recip_cs[:, st:st + 1],
                                         cs_parts[:, st, :],
                                         axis=mybir.AxisListType.X)
                    nc.vector.reciprocal(recip_cs[:, st:st + 1],
                                         recip_cs[:, st:st + 1])
                    dt_ps = pdt.tile([P, D], F32, tag="pdt", name=f"dt{st}")
                    for lt in range(NLT):
                        nc.tensor.matmul(dt_ps, E[lt][:, st * P:(st + 1) * P],
                                         Vlat[lt],
                                         start=(lt == 0), stop=(lt == NLT - 1))
                    tk_ps = ptk.tile([P, D], F32R, tag="ptk", name=f"tk{st}")
                    for j in range(NDT):
                        nc.tensor.transpose(tk_ps[:, j * P:(j + 1) * P],
                                            tokT[j][:, st * P:(st + 1) * P], ident)
                    tok_sb = toksb.tile([P, D], F32, tag="toksb", name=f"toksb{st}")
                    nc.scalar.copy(tok_sb, tk_ps)
                    ut = outp.tile([P, D], F32, tag="outt", name=f"ut{st}")
                    nc.vector.scalar_tensor_tensor(
                        ut, dt_ps, recip_cs[:, st:st + 1], tok_sb,
                        op0=mybir.AluOpType.mult, op1=mybir.AluOpType.add)
                    nc.sync.dma_start(d["out"][L + st * P:L + (st + 1) * P, :], ut)
                    if st < NST - 1:
                        for k in range(4):
                            nc.tensor.matmul(dl_ps[k], e2[:, k * P:(k + 1) * P],
                                             Vtok[st],
                                             start=(st == 0), stop=False)
            for k, lt in enumerate(lts):
                ul = outp.tile([P, D], F32, tag="outt", name=f"ul{lt}")
                nc.vector.scalar_tensor_tensor(
                    ul, dl_ps[k], recip_rs[:, lt:lt + 1], lf_tiles[lt],
                    op0=mybir.AluOpType.mult, op1=mybir.AluOpType.add)
                nc.sync.dma_start(d["out"][lt * P:(lt + 1) * P, :], ul)


def _build_nc(reps=1):
    nc = bacc.Bacc("TRN2", target_bir_lowering=False)
    d = {
        "x": nc.dram_tensor("x", (S2, KC), F32R, kind="ExternalInput"),
        "latr": nc.dram_tensor("latents_r", (L, D), F32R, kind="ExternalInput"),
        "lat": nc.dram_tensor("latents", (L, D), F32, kind="ExternalInput"),
        "wct": nc.dram_tensor("wct", (KC, D), F32R, kind="ExternalInput"),
        "wlat": nc.dram_tensor("wlat", (D, D), F32R, kind="ExternalInput"),
        "wtok": nc.dram_tensor("wtok", (D, D), F32R, kind="ExternalInput"),
        "wvlat": nc.dram_tensor("wvlat", (D, D), F32R, kind="ExternalInput"),
        "wvtok": nc.dram_tensor("wvtok", (D, D), F32R, kind="ExternalInput"),
        "cb": nc.dram_tensor("cb", (P, NDT), F32, kind="ExternalInput"),
        "ident": nc.dram_tensor("ident", (P, P), F32R, kind="ExternalInput"),
        "out": nc.dram_tensor("out", (L + S2, D), F32, kind="ExternalOutput"),
    }
    with tile.TileContext(nc) as tc:
        if reps == 1:
            with ExitStack() as top:
                _emit_body(nc, tc, d, top)
        else:
            with tc.For_i(0, reps, 1):
                with ExitStack() as top:
                    _emit_body(nc, tc, d, top)
    nc.compile()
    return nc


_CACHE = {}


def _get_nc(reps=1):
    key = ("nc", reps)
    if key not in _CACHE:
        _CACHE[key] = _build_nc(reps)
    return _CACHE[key]


def kernel(latents, tokens, W_lat, W_tok, W_vlat, W_vtok, conv_w, conv_b):
    latents = np.ascontiguousarray(np.asarray(latents, dtype=np.float32))
    tokens = np.ascontiguousarray(np.asarray(tokens, dtype=np.float32))
    B = latents.shape[0]
    assert B == NCORES and latents.shape == (B, L, D) and tokens.shape == (B, S2 * 5, 256)

    wct = np.ascontiguousarray(np.asarray(conv_w, np.float32).transpose(2, 1, 0).reshape(KC, D))
    wlat = np.ascontiguousarray(np.asarray(W_lat, np.float32).T)
    wtok = np.ascontiguousarray(np.asarray(W_tok, np.float32).T)
    wvlat = np.ascontiguousarray(np.asarray(W_vlat, np.float32).T)
    wvtok = np.ascontiguousarray(np.asarray(W_vtok, np.float32).T)
    cb = np.ascontiguousarray(np.asarray(conv_b, np.float32).reshape(NDT, P).T)
    ident128 = np.eye(P, dtype=np.float32)

    nc = _get_nc()
    in_maps = []
    for b in range(B):
        xb = np.ascontiguousarray(tokens[b].reshape(S2, KC))
        in_maps.append({
            "x": xb,
            "latents_r": latents[b],
            "latents": latents[b],
            "wct": wct,
            "wlat": wlat,
            "wtok": wtok,
            "wvlat": wvlat,
            "wvtok": wvtok,
            "cb": cb,
            "ident": ident128,
        })
    res = run_bass_kernel_spmd(nc, in_maps, core_ids=list(range(NCORES)))
    out = np.stack([res.results[b]["out"] for b in range(B)])
    updated_latents = out[:, :L, :]
    updated_tokens = out[:, L:, :]
    return (updated_latents, updated_tokens, out)
